# revision 1
# baseline (speedup 1.0000x reference)
"""Trainium2 Bass kernel for nn_FCN_81621558493619.

Computation: 3-layer MLP (mat-vec, 8192->8192->8192->16394) + box-filter +
linear interpolation + Fermi-window trapezoid integrals.

Strategy (8 NeuronCores, SPMD + collectives):
  - Tensor-parallel over output rows of W1/W2/W3 (1024/1024/2176 rows per core,
    W3 zero-padded 16394->17408). Weights cast to bf16 (halves HBM traffic;
    ~4e-3 rel err).
  - Mat-vec per 128-row tile: elementwise mult (DVE + GPSIMD, split) then
    free-axis reduce (ACT accum / DVE tensor_reduce, split). AllGather (bf16)
    of activations between layers.
  - Interpolation: bf16 pair table T2[m] = (S10[m], S10[m+1]) where
    S10 = 10-point box sum of Q; one chunked gpsimd ap_gather (d=2) per core;
    DVE computes indices/fracs and the blended, weighted trapz reductions.
    The 0.1 box-filter scale and the trapz weights are folded into per-j
    coefficient vectors a_j (for I1) and b_j (for -I2) computed on host from
    Wc only.
"""
import numpy as np
import ml_dtypes

import concourse.bacc as bacc
import concourse.mybir as mybir
from concourse import tile
from concourse.bass_utils import run_bass_kernel_spmd

F32 = mybir.dt.float32
BF16 = mybir.dt.bfloat16
I16 = mybir.dt.int16
I32 = mybir.dt.int32

SIZE = 8192
NCORE = 8
SH1 = SIZE // NCORE            # 1024 rows of W1/W2 per core
T1 = SH1 // 128                # 8 tiles
NROW3 = 2 * SIZE + 10          # 16394
NROW3P = 17408                 # padded to 8*17*128
SH3 = NROW3P // NCORE          # 2176
T3 = SH3 // 128                # 17 tiles
NJ = 101
NU = 128                       # samples per Q7 group
SAMP_PER_CORE = SIZE // NCORE  # 1024
# j-chunks for the gather/tail pipeline (must keep num_idxs % 4 == 0 -> any
# j count works since 8 slots/j -> idxs per chunk = 128*jc, slots = 8*jc)
J_CHUNKS = [(0, 25), (25, 50), (50, 75), (75, 101)]

# engine split for the 33 mult and 33 reduce passes (per-core)
N_TILES = T1 + T1 + T3  # 33


def _mult_engine(i):
    # ~1/3 of mults on gpsimd (19.9us/pass) vs DVE (12us/pass)
    return "gp" if i % 3 == 2 else "dve"


def _reduce_engine(i):
    # ~2/3 of reduces on ACT (13.2us/pass) vs DVE tensor_reduce (12us/pass)
    return "dve" if i % 3 == 2 else "act"


def build_nc():
    nc = bacc.Bacc("TRN2", target_bir_lowering=False, debug=False,
                   num_devices=NCORE)

    # ---- per-core external inputs ----
    w1 = nc.dram_tensor("w1", [SH1, SIZE], BF16, kind="ExternalInput")
    w2 = nc.dram_tensor("w2", [SH1, SIZE], BF16, kind="ExternalInput")
    w3 = nc.dram_tensor("w3", [SH3, SIZE], BF16, kind="ExternalInput")
    b1s = nc.dram_tensor("b1s", [128, T1], F32, kind="ExternalInput")
    b2s = nc.dram_tensor("b2s", [128, T1], F32, kind="ExternalInput")
    b3s = nc.dram_tensor("b3s", [128, T3], F32, kind="ExternalInput")
    xbf = nc.dram_tensor("xbf", [SIZE], BF16, kind="ExternalInput")
    xg = nc.dram_tensor("xg", [128, 8], F32, kind="ExternalInput")
    xu = nc.dram_tensor("xu", [128, NU], F32, kind="ExternalInput")
    sjb = nc.dram_tensor("sjb", [128, NJ], F32, kind="ExternalInput")
    ab = nc.dram_tensor("ab", [128, NJ], BF16, kind="ExternalInput")
    bb = nc.dram_tensor("bb", [128, NJ], BF16, kind="ExternalInput")
    out = nc.dram_tensor("out", [2048], F32, kind="ExternalOutput")

    RG = [list(range(NCORE))]

    with tile.TileContext(nc) as tc:
        with tc.tile_pool(name="dram", bufs=1, space="DRAM") as dpool, \
             tc.tile_pool(name="small", bufs=1) as sp:
            # persistent small tiles
            xgt = sp.tile([128, 8], F32)
            xut = sp.tile([128, NU], F32)
            sjt = sp.tile([128, NJ], F32)
            abt = sp.tile([128, NJ], BF16)
            bbt = sp.tile([128, NJ], BF16)
            i1acc = sp.tile([128, NU], F32)
            i2acc = sp.tile([128, NU], F32)
            idx16 = sp.tile([128, NJ * 8], I16)
            frb = sp.tile([128, NJ * NU], BF16)   # frac, bf16, (j,u) layout
            nc.sync.dma_start(xgt[:], xg[:])
            nc.sync.dma_start(xut[:], xu[:])
            nc.sync.dma_start(sjt[:], sjb[:])
            nc.sync.dma_start(abt[:], ab[:])
            nc.sync.dma_start(bbt[:], bb[:])
            nc.vector.memset(i1acc[:], 0.0)
            nc.vector.memset(i2acc[:], 0.0)

            # DRAM bounce buffers
            cc_in1 = dpool.tile([SH1], BF16, name="cc_in1")
            cc_out1 = dpool.tile([SIZE], BF16, name="cc_out1")
            cc_in2 = dpool.tile([SH1], BF16, name="cc_in2")
            cc_out2 = dpool.tile([SIZE], BF16, name="cc_out2")
            q_in = dpool.tile([SH3], F32, name="q_in")
            q_full = dpool.tile([NROW3P], F32, name="q_full")
            t2d = dpool.tile([2 * 16384], BF16, name="t2d")

            # ---------------- index/frac prep (depends only on x) ---------
            with tc.tile_pool(name="prep", bufs=1) as pp:
                for (j0, j1) in J_CHUNKS:
                    jc = j1 - j0
                    wq = jc * NU
                    sx = pp.tile([128, wq], F32, tag="sx", name="sx")
                    pm = pp.tile([128, wq], F32, tag="pm", name="pm")
                    i0i = pp.tile([128, wq], I32, tag="i0i", name="i0i")
                    i0f = pp.tile([128, wq], F32, tag="i0f", name="i0f")
                    # sx[p, (j,u)] = s_j * x_u   (x replicated per group)
                    nc.vector.tensor_tensor(
                        out=sx[:],
                        in0=xut[:].unsqueeze(1).to_broadcast([128, jc, NU]),
                        in1=sjt[:, j0:j1].unsqueeze(2).to_broadcast([128, jc, NU]),
                        op=mybir.AluOpType.mult)
                    nc.vector.tensor_scalar_add(pm[:], sx[:], 8191.5)
                    nc.vector.tensor_copy(i0i[:], pm[:])  # HW cast rounds -> floor
                    # i0f = min(i0, 16383) - 8192  (f32)
                    nc.vector.tensor_scalar(
                        out=i0f[:], in0=i0i[:], scalar1=16383, scalar2=8192,
                        op0=mybir.AluOpType.min, op1=mybir.AluOpType.subtract)
                    # frac = sx - i0f  (bf16)
                    nc.vector.tensor_tensor(
                        out=frb[:, j0 * NU:j1 * NU], in0=sx[:], in1=i0f[:],
                        op=mybir.AluOpType.subtract)

                    # compact index path for the gather
                    wg = jc * 8
                    sxg = pp.tile([128, wg], F32, tag="sxg", name="sxg")
                    pmg = pp.tile([128, wg], F32, tag="pmg", name="pmg")
                    i0g = pp.tile([128, wg], I32, tag="i0g", name="i0g")
                    nc.vector.tensor_tensor(
                        out=sxg[:],
                        in0=xgt[:].unsqueeze(1).to_broadcast([128, jc, 8]),
                        in1=sjt[:, j0:j1].unsqueeze(2).to_broadcast([128, jc, 8]),
                        op=mybir.AluOpType.mult)
                    nc.vector.tensor_scalar_add(pmg[:], sxg[:], 8191.5)
                    nc.vector.tensor_copy(i0g[:], pmg[:])
                    nc.vector.tensor_scalar(
                        out=idx16[:, j0 * 8:j1 * 8], in0=i0g[:], scalar1=16383,
                        scalar2=None, op0=mybir.AluOpType.min)

            # ---------------- MLP ----------------
            tile_idx = 0

            def matvec_layer(wdram, ntiles, xb_t, y_t, pools):
                nonlocal tile_idx
                wpool, ppool, dumpb = pools
                for t in range(ntiles):
                    wt = wpool.tile([128, SIZE], BF16, tag="w", name=f"w_{tile_idx}")
                    nc.sync.dma_start(wt[:], wdram[t * 128:(t + 1) * 128, :])
                    pr = ppool.tile([128, SIZE], BF16, tag="p",
                                    name=f"pr_{tile_idx}")
                    if _mult_engine(tile_idx) == "gp":
                        nc.gpsimd.tensor_tensor(out=pr[:], in0=wt[:], in1=xb_t[:],
                                                op=mybir.AluOpType.mult)
                    else:
                        nc.vector.tensor_tensor(out=pr[:], in0=wt[:], in1=xb_t[:],
                                                op=mybir.AluOpType.mult)
                    if _reduce_engine(tile_idx) == "act":
                        nc.scalar.activation(
                            out=dumpb[:], in_=pr[:],
                            func=mybir.ActivationFunctionType.Copy,
                            accum_out=y_t[:, t:t + 1])
                    else:
                        nc.vector.tensor_reduce(
                            out=y_t[:, t:t + 1], in_=pr[:],
                            axis=mybir.AxisListType.X, op=mybir.AluOpType.add)
                    tile_idx += 1

            with tc.tile_pool(name="mlp_w", bufs=3) as wpool, \
                 tc.tile_pool(name="mlp_p", bufs=3) as ppool, \
                 tc.tile_pool(name="mlp_misc", bufs=1) as mp:
                dumpb = mp.tile([128, SIZE], BF16)
                xb1 = mp.tile([128, SIZE], BF16, tag="xb", name="xb1", bufs=3)
                y1 = mp.tile([128, T1], F32)
                b1t = mp.tile([128, T1], F32)
                h1 = mp.tile([128, T1], F32)
                h1b = mp.tile([128, T1], BF16)
                nc.sync.dma_start(b1t[:], b1s[:])
                nc.sync.dma_start(
                    xb1[:], xbf.ap()[None, :].to_broadcast([128, SIZE]))
                matvec_layer(w1, T1, xb1, y1, (wpool, ppool, dumpb))
                # h1 = relu(y1 + b1) -> bf16
                nc.vector.tensor_tensor(out=h1[:], in0=y1[:], in1=b1t[:],
                                        op=mybir.AluOpType.add)
                nc.vector.tensor_scalar_max(h1b[:], h1[:], 0.0)
                nc.sync.dma_start(cc_in1[:].rearrange("(t p) -> p t", p=128),
                                  h1b[:])
                nc.gpsimd.collective_compute(
                    "AllGather", mybir.AluOpType.bypass, replica_groups=RG,
                    ins=[cc_in1.opt()], outs=[cc_out1.opt()])

                xb2 = mp.tile([128, SIZE], BF16, tag="xb", name="xb2", bufs=3)
                y2 = mp.tile([128, T1], F32)
                b2t = mp.tile([128, T1], F32)
                h2 = mp.tile([128, T1], F32)
                h2b = mp.tile([128, T1], BF16)
                nc.sync.dma_start(b2t[:], b2s[:])
                nc.sync.dma_start(
                    xb2[:], cc_out1[:][None, :].to_broadcast([128, SIZE]))
                matvec_layer(w2, T1, xb2, y2, (wpool, ppool, dumpb))
                nc.vector.tensor_tensor(out=h2[:], in0=y2[:], in1=b2t[:],
                                        op=mybir.AluOpType.add)
                nc.vector.tensor_scalar_max(h2b[:], h2[:], 0.0)
                nc.sync.dma_start(cc_in2[:].rearrange("(t p) -> p t", p=128),
                                  h2b[:])
                nc.gpsimd.collective_compute(
                    "AllGather", mybir.AluOpType.bypass, replica_groups=RG,
                    ins=[cc_in2.opt()], outs=[cc_out2.opt()])

                xb3 = mp.tile([128, SIZE], BF16, tag="xb", name="xb3", bufs=3)
                y3 = mp.tile([128, T3], F32)
                b3t = mp.tile([128, T3], F32)
                q_sb = mp.tile([128, T3], F32)
                nc.sync.dma_start(b3t[:], b3s[:])
                nc.sync.dma_start(
                    xb3[:], cc_out2[:][None, :].to_broadcast([128, SIZE]))
                matvec_layer(w3, T3, xb3, y3, (wpool, ppool, dumpb))
                nc.vector.tensor_tensor(out=q_sb[:], in0=y3[:], in1=b3t[:],
                                        op=mybir.AluOpType.add)
                nc.sync.dma_start(q_in[:].rearrange("(t p) -> p t", p=128),
                                  q_sb[:])
                nc.gpsimd.collective_compute(
                    "AllGather", mybir.AluOpType.bypass, replica_groups=RG,
                    ins=[q_in.opt()], outs=[q_full.opt()])

            # ---------------- box sum + pair table ----------------
            with tc.tile_pool(name="sig", bufs=1) as gp:
                qov = gp.tile([128, 144], F32)
                sig = gp.tile([128, 129], F32)
                # partition p holds Q[128p .. 128p+143] (overlapping reads)
                from concourse.ap import AP as _AP
                qf_ap = q_full[:]
                nc.sync.dma_start(
                    qov[:], _AP(qf_ap.tensor, 0, [[128, 128], [1, 144]]))
                nc.vector.tensor_copy(sig[:], qov[:, 0:129])
                for d in range(1, 10):
                    nc.vector.tensor_tensor(out=sig[:], in0=sig[:],
                                            in1=qov[:, d:d + 129],
                                            op=mybir.AluOpType.add)
                # pair table: interleave in SBUF (bf16) then contiguous DMA out
                pair_sb = gp.tile([128, 256], BF16)
                pv = pair_sb[:].rearrange("p (f c) -> p f c", f=128, c=2)
                nc.vector.tensor_copy(pv[:, :, 0], sig[:, 0:128])
                nc.vector.tensor_copy(pv[:, :, 1], sig[:, 1:129])
                nc.sync.dma_start(
                    t2d[:].rearrange("(p f) -> p f", p=128, f=256), pair_sb[:])

            # ---------------- gather + blend + integrate ----------------
            with tc.tile_pool(name="interp", bufs=1) as ip:
                tab2 = ip.tile([128, 2 * 16384], BF16)
                nc.sync.dma_start(
                    tab2[:], t2d[:][None, :].to_broadcast([128, 2 * 16384]))
                for ci, (j0, j1) in enumerate(J_CHUNKS):
                    jc = j1 - j0
                    wq = jc * NU
                    gab = ip.tile([128, 2 * wq], BF16, tag="gab", bufs=2,
                                  name=f"gab{ci}")
                    nc.gpsimd.ap_gather(
                        gab[:], tab2[:], idx16[:, j0 * 8:j1 * 8],
                        channels=128, num_elems=16384, d=2, num_idxs=wq)
                    g0 = gab[:].rearrange("p (q c) -> p q c", c=2)[:, :, 0]
                    g1 = gab[:].rearrange("p (q c) -> p q c", c=2)[:, :, 1]
                    dd = ip.tile([128, wq], BF16, tag="dd", bufs=2,
                                 name=f"dd{ci}")
                    sS = ip.tile([128, wq], BF16, tag="ss", bufs=2,
                                 name=f"ss{ci}")
                    pa = ip.tile([128, wq], BF16, tag="pa", bufs=2,
                                 name=f"pa{ci}")
                    i1p = ip.tile([128, NU], F32, tag="i1p", bufs=2,
                                  name=f"i1p{ci}")
                    i2p = ip.tile([128, NU], F32, tag="i2p", bufs=2,
                                  name=f"i2p{ci}")
                    frc = frb[:, j0 * NU:j1 * NU]
                    nc.vector.tensor_tensor(out=dd[:], in0=g1, in1=g0,
                                            op=mybir.AluOpType.subtract)
                    nc.vector.tensor_tensor(out=dd[:], in0=frc, in1=dd[:],
                                            op=mybir.AluOpType.mult)
                    nc.vector.tensor_tensor(out=sS[:], in0=g0, in1=dd[:],
                                            op=mybir.AluOpType.add)
                    abv = abt[:, j0:j1].unsqueeze(2).to_broadcast([128, jc, NU])
                    bbv = bbt[:, j0:j1].unsqueeze(2).to_broadcast([128, jc, NU])
                    nc.vector.tensor_tensor(out=pa[:], in0=sS[:], in1=abv,
                                            op=mybir.AluOpType.mult)
                    nc.vector.tensor_reduce(
                        out=i1p[:], in_=pa[:].rearrange("p (j u) -> p u j",
                                                        j=jc, u=NU),
                        axis=mybir.AxisListType.X, op=mybir.AluOpType.add)
                    nc.vector.tensor_tensor(out=i1acc[:], in0=i1acc[:],
                                            in1=i1p[:],
                                            op=mybir.AluOpType.add)
                    nc.vector.tensor_tensor(out=pa[:], in0=sS[:], in1=bbv,
                                            op=mybir.AluOpType.mult)
                    nc.vector.tensor_reduce(
                        out=i2p[:], in_=pa[:].rearrange("p (j u) -> p u j",
                                                        j=jc, u=NU),
                        axis=mybir.AxisListType.X, op=mybir.AluOpType.add)
                    nc.vector.tensor_tensor(out=i2acc[:], in0=i2acc[:],
                                            in1=i2p[:],
                                            op=mybir.AluOpType.add)

                # I2 = x_i * sum_j b_j S_ij  (x does not cancel for I2)
                nc.vector.tensor_tensor(out=i2acc[:], in0=i2acc[:],
                                        in1=xut[:], op=mybir.AluOpType.mult)
                # outputs: row r=0 of each 16-partition group
                nc.sync.dma_start(
                    out[0:1024].rearrange("(g u) -> g u", g=8, u=NU),
                    i1acc[0:128:16, :])
                nc.sync.dma_start(
                    out[1024:2048].rearrange("(g u) -> g u", g=8, u=NU),
                    i2acc[0:128:16, :])

    nc.compile()
    return nc


_NC_CACHE = {}


def _get_nc():
    if "nc" not in _NC_CACHE:
        _NC_CACHE["nc"] = build_nc()
    return _NC_CACHE["nc"]


def _host_prep(x, Wc, W1, b1, W2, b2, W3, b3):
    bf = ml_dtypes.bfloat16
    x = np.asarray(x, np.float32)
    Wcf = np.float64(np.asarray(Wc).item())
    # t grid and Fermi window (match reference's fp32 values closely)
    t = (np.linspace(-1.0, 1.0, NJ, dtype=np.float32)
         * np.float32(Wcf)).astype(np.float32)
    step = np.float32(Wcf) / np.float32(SIZE)
    s = (t / step).astype(np.float32)           # pos = x*s + SIZE
    eu = np.exp(t.astype(np.float64))
    g = eu / (eu + 1.0) ** 2                     # fermi window * x (x cancels)
    d = np.diff(t.astype(np.float64))            # actual fp32 grid deltas
    wtrap = np.zeros(NJ)
    wtrap[:-1] += 0.5 * d
    wtrap[1:] += 0.5 * d
    a = (0.1 * g * wtrap).astype(np.float32)     # 0.1 = box-filter fold
    b = (-0.1 * t.astype(np.float64) * g * wtrap).astype(np.float32)

    W3p = np.zeros((NROW3P, SIZE), dtype=np.float32)
    W3p[:NROW3] = W3
    b3p = np.zeros(NROW3P, dtype=np.float32)
    b3p[:NROW3] = b3

    in_maps = []
    for c in range(NCORE):
        xc = x[c * SAMP_PER_CORE:(c + 1) * SAMP_PER_CORE]
        xg = np.zeros((128, 8), np.float32)
        xu = np.zeros((128, NU), np.float32)
        for gidx in range(8):
            seg = xc[gidx * 128:(gidx + 1) * 128]
            xu[16 * gidx:16 * (gidx + 1), :] = seg[None, :]
            xg[16 * gidx:16 * gidx + 16, :] = seg.reshape(8, 16).T
        in_maps.append({
            "w1": np.ascontiguousarray(
                W1[c * SH1:(c + 1) * SH1]).astype(bf),
            "w2": np.ascontiguousarray(
                W2[c * SH1:(c + 1) * SH1]).astype(bf),
            "w3": np.ascontiguousarray(
                W3p[c * SH3:(c + 1) * SH3]).astype(bf),
            "b1s": np.ascontiguousarray(
                b1[c * SH1:(c + 1) * SH1].reshape(T1, 128).T),
            "b2s": np.ascontiguousarray(
                b2[c * SH1:(c + 1) * SH1].reshape(T1, 128).T),
            "b3s": np.ascontiguousarray(
                b3p[c * SH3:(c + 1) * SH3].reshape(T3, 128).T),
            "xbf": x.astype(bf),
            "xg": xg,
            "xu": xu,
            "sjb": np.broadcast_to(s, (128, NJ)).copy(),
            "ab": np.broadcast_to(a.astype(bf), (128, NJ)).copy(),
            "bb": np.broadcast_to(b.astype(bf), (128, NJ)).copy(),
        })
    return in_maps


def kernel(x, Wc, W1, b1, W2, b2, W3, b3, _trace=False, _res_box=None):
    nc = _get_nc()
    in_maps = _host_prep(np.asarray(x), Wc, np.asarray(W1), np.asarray(b1),
                         np.asarray(W2), np.asarray(b2), np.asarray(W3),
                         np.asarray(b3))
    res = run_bass_kernel_spmd(nc, in_maps, core_ids=list(range(NCORE)),
                               trace=_trace)
    if _res_box is not None:
        _res_box.append(res)
    i1 = np.concatenate([r["out"][0:1024] for r in res.results])
    i2 = np.concatenate([r["out"][1024:2048] for r in res.results])
    return np.concatenate([i1, i2]).astype(np.float32)



# revision 2
# speedup vs baseline: 1.1240x; 1.1240x over previous
"""Trainium2 Bass kernel for nn_FCN_81621558493619 (v2: TensorEngine MLP).

Computation: 3-layer MLP (mat-vec, 8192->8192->8192->16394) + box-filter +
linear interpolation + Fermi-window trapezoid integrals.

Strategy (8 NeuronCores, SPMD + collectives):
  - Mat-vecs on the TensorEngine: stationary = x k-chunk [128,1] (LDWEIGHTS
    ~1 cycle), moving = W.T tile [128,512] bf16 streamed from SBUF at
    128 elem/cycle.  PSUM rows [1,512] accumulate over 64 k-chunks.
  - Sharding: L1 row-shard (1024 rows/core), L2 column-shard (local h1
    shard as stationary -> no collective between L1/L2), partial y2
    AllReduce'd in 4 segments (overlapped with L2/L3), L3 row-shard
    (2176 rows/core of zero-padded 17408), AllGather Q.
  - Weights pre-transposed + tiled on host so each DMA chunk is a
    [128, contig] slice (1-2 MB per dma_start, ~line rate).
  - Interpolation stage identical to v1 baseline: bf16 pair table
    T2[m] = (S10[m], S10[m+1]), chunked gpsimd ap_gather (d=2), DVE
    blend + weighted trapz reductions.  Index/frac prep hides under the
    MLP (DVE is otherwise idle now).
"""
import numpy as np
import ml_dtypes

import concourse.bacc as bacc
import concourse.mybir as mybir
from concourse import tile
from concourse.bass_utils import run_bass_kernel_spmd

F32 = mybir.dt.float32
BF16 = mybir.dt.bfloat16
I16 = mybir.dt.int16
I32 = mybir.dt.int32

SIZE = 8192
NCORE = 8
SH1 = SIZE // NCORE            # 1024 rows of W1 (and cols of W2) per core
NROW3 = 2 * SIZE + 10          # 16394
NROW3P = 17408                 # padded to 8*2176
SH3 = NROW3P // NCORE          # 2176
NJ = 101
NU = 128                       # samples per group
SAMP_PER_CORE = SIZE // NCORE  # 1024
J_CHUNKS = [(0, 25), (25, 50), (50, 75), (75, 101)]

KC1 = 64                       # k-chunks of 128 in L1/L3 contraction
KC2 = 8                        # k-chunks in L2 contraction (local 1024)
MB1 = 2                        # L1 m-blocks of 512
NB2 = 16                       # L2 n-blocks of 512
MB3W = [512, 512, 512, 512, 128]   # L3 m-block widths (2176 total)
MB3OFF = [0, 512, 1024, 1536, 2048]
W3COL = KC1 * SH3              # per-partition w3 row length: 139264
SUBKC = 16                     # k-chunks per DMA chunk (2 MB for 512-wide)


def build_nc():
    nc = bacc.Bacc("TRN2", target_bir_lowering=False, debug=False,
                   num_devices=NCORE)

    # ---- per-core external inputs ----
    # weight layouts (host-pretransposed, see _host_prep):
    #   w1[p, mb*32768 + kc*512 + n] = W1sh[mb*512+n, kc*128+p]
    #   w2[p, nb*4096  + kc*512 + n] = W2blk[nb*512+n, kc*128+p]
    #   w3[p, mboff*64 + kc*wmb + n] = W3sh[mb*512+n, kc*128+p]
    w1 = nc.dram_tensor("w1", [128, MB1 * KC1 * 512], BF16, kind="ExternalInput")
    w2 = nc.dram_tensor("w2", [128, NB2 * KC2 * 512], BF16, kind="ExternalInput")
    w3 = nc.dram_tensor("w3", [128, W3COL], BF16, kind="ExternalInput")
    b1d = nc.dram_tensor("b1d", [SH1], F32, kind="ExternalInput")
    b2d = nc.dram_tensor("b2d", [128, KC1], F32, kind="ExternalInput")
    b3d = nc.dram_tensor("b3d", [SH3], F32, kind="ExternalInput")
    xbf = nc.dram_tensor("xbf", [SIZE], BF16, kind="ExternalInput")
    xg = nc.dram_tensor("xg", [128, 8], F32, kind="ExternalInput")
    xu = nc.dram_tensor("xu", [128, NU], F32, kind="ExternalInput")
    sjb = nc.dram_tensor("sjb", [128, NJ], F32, kind="ExternalInput")
    ab = nc.dram_tensor("ab", [128, NJ], BF16, kind="ExternalInput")
    bb = nc.dram_tensor("bb", [128, NJ], BF16, kind="ExternalInput")
    out = nc.dram_tensor("out", [2048], F32, kind="ExternalOutput")

    RG = [list(range(NCORE))]

    with tile.TileContext(nc) as tc:
        with tc.tile_pool(name="dram", bufs=1, space="DRAM") as dpool, \
             tc.tile_pool(name="small", bufs=1) as sp:
            # persistent small tiles
            xgt = sp.tile([128, 8], F32)
            xut = sp.tile([128, NU], F32)
            sjt = sp.tile([128, NJ], F32)
            abt = sp.tile([128, NJ], BF16)
            bbt = sp.tile([128, NJ], BF16)
            i1acc = sp.tile([128, NU], F32)
            i2acc = sp.tile([128, NU], F32)
            idx16 = sp.tile([128, NJ * 8], I16)
            frb = sp.tile([128, NJ * NU], BF16)   # frac, bf16, (j,u) layout

            nc.sync.dma_start(xgt[:], xg[:])
            nc.sync.dma_start(xut[:], xu[:])
            nc.sync.dma_start(sjt[:], sjb[:])
            nc.sync.dma_start(abt[:], ab[:])
            nc.sync.dma_start(bbt[:], bb[:])
            nc.vector.memset(i1acc[:], 0.0)
            nc.vector.memset(i2acc[:], 0.0)

            # DRAM bounce buffers
            h1dr = dpool.tile([SH1], BF16, name="h1dr")
            ar_in = [dpool.tile([2048], F32, name=f"ar_in{s}") for s in range(4)]
            ar_out = [dpool.tile([2048], F32, name=f"ar_out{s}") for s in range(4)]
            q_in = dpool.tile([SH3], F32, name="q_in")
            q_full = dpool.tile([NROW3P], F32, name="q_full")
            t2d = dpool.tile([2 * 16384], BF16, name="t2d")

            with tc.tile_pool(name="prep", bufs=1) as pp, \
                 tc.tile_pool(name="mlp_w", bufs=4) as wpool, \
                 tc.tile_pool(name="mlp_m", bufs=1) as mm:
                # MLP-scoped tiles ([1, N] tiles still cost N*dtype bytes of
                # every partition's budget -- keep them out of the
                # persistent pool so the interp pool can fit later)
                xsb1 = mm.tile([128, KC1], BF16)
                xsb2 = mm.tile([128, KC2], BF16)
                xsb3 = mm.tile([128, KC1], BF16)
                b1sb = mm.tile([1, SH1], F32)
                b2sb = mm.tile([128, KC1], F32)
                b3sb = mm.tile([1, SH3], F32)
                h1f = mm.tile([1, SH1], F32)
                h1b = mm.tile([1, SH1], BF16)
                y2sb = mm.tile([1, SIZE], F32)
                qsb = mm.tile([1, SH3], F32)
                nc.sync.dma_start(
                    xsb1[:], xbf.ap().rearrange("(kc p) -> p kc", p=128))
                nc.sync.dma_start(
                    b1sb[:], b1d.ap().rearrange("(o f) -> o f", o=1))
                nc.sync.dma_start(b2sb[:], b2d[:])
                nc.sync.dma_start(
                    b3sb[:], b3d.ap().rearrange("(o f) -> o f", o=1))
                # ------------ index/frac prep (depends only on x) ---------
                for (j0, j1) in J_CHUNKS:
                    jc = j1 - j0
                    wq = jc * NU
                    sx = pp.tile([128, wq], F32, tag="sx", name="sx")
                    pm = pp.tile([128, wq], F32, tag="pm", name="pm")
                    i0i = pp.tile([128, wq], I32, tag="i0i", name="i0i")
                    i0f = pp.tile([128, wq], F32, tag="i0f", name="i0f")
                    # sx[p, (j,u)] = s_j * x_u   (x replicated per group)
                    nc.vector.tensor_tensor(
                        out=sx[:],
                        in0=xut[:].unsqueeze(1).to_broadcast([128, jc, NU]),
                        in1=sjt[:, j0:j1].unsqueeze(2).to_broadcast([128, jc, NU]),
                        op=mybir.AluOpType.mult)
                    nc.vector.tensor_scalar_add(pm[:], sx[:], 8191.5)
                    nc.vector.tensor_copy(i0i[:], pm[:])  # HW cast rounds -> floor
                    # i0f = min(i0, 16383) - 8192  (f32)
                    nc.vector.tensor_scalar(
                        out=i0f[:], in0=i0i[:], scalar1=16383, scalar2=8192,
                        op0=mybir.AluOpType.min, op1=mybir.AluOpType.subtract)
                    # frac = sx - i0f  (bf16)
                    nc.vector.tensor_tensor(
                        out=frb[:, j0 * NU:j1 * NU], in0=sx[:], in1=i0f[:],
                        op=mybir.AluOpType.subtract)

                    # compact index path for the gather
                    wg = jc * 8
                    sxg = pp.tile([128, wg], F32, tag="sxg", name="sxg")
                    pmg = pp.tile([128, wg], F32, tag="pmg", name="pmg")
                    i0g = pp.tile([128, wg], I32, tag="i0g", name="i0g")
                    nc.vector.tensor_tensor(
                        out=sxg[:],
                        in0=xgt[:].unsqueeze(1).to_broadcast([128, jc, 8]),
                        in1=sjt[:, j0:j1].unsqueeze(2).to_broadcast([128, jc, 8]),
                        op=mybir.AluOpType.mult)
                    nc.vector.tensor_scalar_add(pmg[:], sxg[:], 8191.5)
                    nc.vector.tensor_copy(i0g[:], pmg[:])
                    nc.vector.tensor_scalar(
                        out=idx16[:, j0 * 8:j1 * 8], in0=i0g[:], scalar1=16383,
                        scalar2=None, op0=mybir.AluOpType.min)

                # ------------------ L1: h1_shard = relu(W1sh @ x + b1sh) --
                with tc.tile_pool(name="ps1", bufs=1, space="PSUM") as ps1p:
                    ps1 = [ps1p.tile([1, 512], F32, tag=f"ps1_{mb}",
                                     name=f"ps1_{mb}") for mb in range(MB1)]
                    for mb in range(MB1):
                        for sub in range(KC1 // SUBKC):
                            wt = wpool.tile([128, SUBKC * 512], BF16, tag="w",
                                            name=f"w1_{mb}_{sub}")
                            off = mb * KC1 * 512 + sub * SUBKC * 512
                            nc.sync.dma_start(
                                wt[:], w1[:, off:off + SUBKC * 512])
                            for j in range(SUBKC):
                                kc = sub * SUBKC + j
                                nc.tensor.matmul(
                                    ps1[mb][:, :],
                                    xsb1[:, kc:kc + 1],
                                    wt[:, j * 512:(j + 1) * 512],
                                    start=(kc == 0), stop=(kc == KC1 - 1))
                        # per-m-block epilogue: bias + relu -> bf16
                        sl = slice(mb * 512, (mb + 1) * 512)
                        nc.vector.tensor_tensor(
                            out=h1f[:, sl], in0=ps1[mb][:, :], in1=b1sb[:, sl],
                            op=mybir.AluOpType.add)
                        nc.vector.tensor_scalar_max(h1b[:, sl], h1f[:, sl], 0.0)
                        nc.sync.dma_start(
                            h1dr[mb * 512:(mb + 1) * 512].rearrange(
                                "(o f) -> o f", o=1), h1b[:, sl])
                # h1 bf16 [1024] -> stationary layout [128, 8]
                nc.sync.dma_start(
                    xsb2[:], h1dr[:].rearrange("(kc p) -> p kc", p=128))

                # ------------------ L2: y2_partial = W2[:, blk] @ h1_shard --
                with tc.tile_pool(name="ps2", bufs=1, space="PSUM") as ps2p:
                    for nb in range(NB2):
                        wt = wpool.tile([128, KC2 * 512], BF16, tag="w",
                                        name=f"w2_{nb}")
                        off = nb * KC2 * 512
                        nc.sync.dma_start(wt[:], w2[:, off:off + KC2 * 512])
                        ps = ps2p.tile([1, 512], F32, tag="ps2", bufs=3,
                                       name=f"ps2_{nb}")
                        for kc in range(KC2):
                            nc.tensor.matmul(
                                ps[:, :], xsb2[:, kc:kc + 1],
                                wt[:, kc * 512:(kc + 1) * 512],
                                start=(kc == 0), stop=(kc == KC2 - 1))
                        nc.scalar.activation(
                            out=y2sb[:, nb * 512:(nb + 1) * 512], in_=ps[:, :],
                            func=mybir.ActivationFunctionType.Copy)
                        if nb % 4 == 3:
                            s = nb // 4
                            nc.sync.dma_start(
                                ar_in[s][:].rearrange("(o f) -> o f", o=1),
                                y2sb[:, s * 2048:(s + 1) * 2048])
                            nc.gpsimd.collective_compute(
                                "AllReduce", mybir.AluOpType.add,
                                replica_groups=RG,
                                ins=[ar_in[s].opt()], outs=[ar_out[s].opt()])
                            # post-AR: + b2, relu -> xsb3 seg (bf16)
                            t32 = pp.tile([128, 16], F32, tag="t32", bufs=2,
                                          name=f"t32_{s}")
                            nc.sync.dma_start(
                                t32[:],
                                ar_out[s][:].rearrange("(kc p) -> p kc",
                                                       p=128))
                            ssl = slice(s * 16, (s + 1) * 16)
                            nc.vector.tensor_tensor(
                                out=t32[:], in0=t32[:], in1=b2sb[:, ssl],
                                op=mybir.AluOpType.add)
                            nc.vector.tensor_scalar_max(
                                xsb3[:, ssl], t32[:], 0.0)

                # ------------------ L3: Q_shard = W3sh @ h2 + b3sh ---------
                with tc.tile_pool(name="ps3", bufs=1, space="PSUM") as ps3p:
                    for mb in range(5):
                        wmb = MB3W[mb]
                        ps = ps3p.tile([1, wmb], F32, tag=f"ps3_{mb}",
                                       name=f"ps3_{mb}")
                        mboff = MB3OFF[mb] * KC1
                        for sub in range(KC1 // SUBKC):
                            wt = wpool.tile([128, SUBKC * wmb], BF16, tag="w",
                                            name=f"w3_{mb}_{sub}")
                            off = mboff + sub * SUBKC * wmb
                            nc.sync.dma_start(
                                wt[:], w3[:, off:off + SUBKC * wmb])
                            for j in range(SUBKC):
                                kc = sub * SUBKC + j
                                nc.tensor.matmul(
                                    ps[:, :], xsb3[:, kc:kc + 1],
                                    wt[:, j * wmb:(j + 1) * wmb],
                                    start=(kc == 0), stop=(kc == KC1 - 1))
                        sl = slice(MB3OFF[mb], MB3OFF[mb] + wmb)
                        nc.vector.tensor_tensor(
                            out=qsb[:, sl], in0=ps[:, :], in1=b3sb[:, sl],
                            op=mybir.AluOpType.add)
                        nc.sync.dma_start(
                            q_in[sl].rearrange("(o f) -> o f", o=1),
                            qsb[:, sl])
                nc.gpsimd.collective_compute(
                    "AllGather", mybir.AluOpType.bypass, replica_groups=RG,
                    ins=[q_in.opt()], outs=[q_full.opt()])

            # ---------------- box sum + pair table ----------------
            with tc.tile_pool(name="sig", bufs=1) as gp:
                qov = gp.tile([128, 144], F32)
                sig = gp.tile([128, 129], F32)
                # partition p holds Q[128p .. 128p+143] (overlapping reads)
                from concourse.ap import AP as _AP
                qf_ap = q_full[:]
                nc.sync.dma_start(
                    qov[:], _AP(qf_ap.tensor, 0, [[128, 128], [1, 144]]))
                nc.vector.tensor_copy(sig[:], qov[:, 0:129])
                for d in range(1, 10):
                    nc.vector.tensor_tensor(out=sig[:], in0=sig[:],
                                            in1=qov[:, d:d + 129],
                                            op=mybir.AluOpType.add)
                # pair table: interleave in SBUF (bf16) then contiguous DMA out
                pair_sb = gp.tile([128, 256], BF16)
                pv = pair_sb[:].rearrange("p (f c) -> p f c", f=128, c=2)
                nc.vector.tensor_copy(pv[:, :, 0], sig[:, 0:128])
                nc.vector.tensor_copy(pv[:, :, 1], sig[:, 1:129])
                nc.sync.dma_start(
                    t2d[:].rearrange("(p f) -> p f", p=128, f=256), pair_sb[:])

            # ---------------- gather + blend + integrate ----------------
            with tc.tile_pool(name="interp", bufs=1) as ip:
                tab2 = ip.tile([128, 2 * 16384], BF16)
                nc.sync.dma_start(
                    tab2[:], t2d[:][None, :].to_broadcast([128, 2 * 16384]))
                for ci, (j0, j1) in enumerate(J_CHUNKS):
                    jc = j1 - j0
                    wq = jc * NU
                    gab = ip.tile([128, 2 * wq], BF16, tag="gab", bufs=2,
                                  name=f"gab{ci}")
                    nc.gpsimd.ap_gather(
                        gab[:], tab2[:], idx16[:, j0 * 8:j1 * 8],
                        channels=128, num_elems=16384, d=2, num_idxs=wq)
                    g0 = gab[:].rearrange("p (q c) -> p q c", c=2)[:, :, 0]
                    g1 = gab[:].rearrange("p (q c) -> p q c", c=2)[:, :, 1]
                    dd = ip.tile([128, wq], BF16, tag="dd", bufs=2,
                                 name=f"dd{ci}")
                    sS = ip.tile([128, wq], BF16, tag="ss", bufs=2,
                                 name=f"ss{ci}")
                    pa = ip.tile([128, wq], BF16, tag="pa", bufs=2,
                                 name=f"pa{ci}")
                    i1p = ip.tile([128, NU], F32, tag="i1p", bufs=2,
                                  name=f"i1p{ci}")
                    i2p = ip.tile([128, NU], F32, tag="i2p", bufs=2,
                                  name=f"i2p{ci}")
                    frc = frb[:, j0 * NU:j1 * NU]
                    nc.vector.tensor_tensor(out=dd[:], in0=g1, in1=g0,
                                            op=mybir.AluOpType.subtract)
                    nc.vector.tensor_tensor(out=dd[:], in0=frc, in1=dd[:],
                                            op=mybir.AluOpType.mult)
                    nc.vector.tensor_tensor(out=sS[:], in0=g0, in1=dd[:],
                                            op=mybir.AluOpType.add)
                    abv = abt[:, j0:j1].unsqueeze(2).to_broadcast([128, jc, NU])
                    bbv = bbt[:, j0:j1].unsqueeze(2).to_broadcast([128, jc, NU])
                    nc.vector.tensor_tensor(out=pa[:], in0=sS[:], in1=abv,
                                            op=mybir.AluOpType.mult)
                    nc.vector.tensor_reduce(
                        out=i1p[:], in_=pa[:].rearrange("p (j u) -> p u j",
                                                        j=jc, u=NU),
                        axis=mybir.AxisListType.X, op=mybir.AluOpType.add)
                    nc.vector.tensor_tensor(out=i1acc[:], in0=i1acc[:],
                                            in1=i1p[:],
                                            op=mybir.AluOpType.add)
                    nc.vector.tensor_tensor(out=pa[:], in0=sS[:], in1=bbv,
                                            op=mybir.AluOpType.mult)
                    nc.vector.tensor_reduce(
                        out=i2p[:], in_=pa[:].rearrange("p (j u) -> p u j",
                                                        j=jc, u=NU),
                        axis=mybir.AxisListType.X, op=mybir.AluOpType.add)
                    nc.vector.tensor_tensor(out=i2acc[:], in0=i2acc[:],
                                            in1=i2p[:],
                                            op=mybir.AluOpType.add)

                # I2 = x_i * sum_j b_j S_ij  (x does not cancel for I2)
                nc.vector.tensor_tensor(out=i2acc[:], in0=i2acc[:],
                                        in1=xut[:], op=mybir.AluOpType.mult)
                # outputs: row r=0 of each 16-partition group
                nc.sync.dma_start(
                    out[0:1024].rearrange("(g u) -> g u", g=8, u=NU),
                    i1acc[0:128:16, :])
                nc.sync.dma_start(
                    out[1024:2048].rearrange("(g u) -> g u", g=8, u=NU),
                    i2acc[0:128:16, :])

    nc.compile()
    return nc


_NC_CACHE = {}


def _get_nc():
    if "nc" not in _NC_CACHE:
        _NC_CACHE["nc"] = build_nc()
    return _NC_CACHE["nc"]


def _host_prep(x, Wc, W1, b1, W2, b2, W3, b3):
    bf = ml_dtypes.bfloat16
    x = np.asarray(x, np.float32)
    Wcf = np.float64(np.asarray(Wc).item())
    # t grid and Fermi window (match reference's fp32 values closely)
    t = (np.linspace(-1.0, 1.0, NJ, dtype=np.float32)
         * np.float32(Wcf)).astype(np.float32)
    step = np.float32(Wcf) / np.float32(SIZE)
    s = (t / step).astype(np.float32)           # pos = x*s + SIZE
    eu = np.exp(t.astype(np.float64))
    g = eu / (eu + 1.0) ** 2                     # fermi window * x (x cancels)
    d = np.diff(t.astype(np.float64))            # actual fp32 grid deltas
    wtrap = np.zeros(NJ)
    wtrap[:-1] += 0.5 * d
    wtrap[1:] += 0.5 * d
    a = (0.1 * g * wtrap).astype(np.float32)     # 0.1 = box-filter fold
    b = (-0.1 * t.astype(np.float64) * g * wtrap).astype(np.float32)

    W1b = np.asarray(W1, np.float32).astype(bf)
    W2b = np.asarray(W2, np.float32).astype(bf)
    W3b = np.asarray(W3, np.float32).astype(bf)
    b3p = np.zeros(NROW3P, dtype=np.float32)
    b3p[:NROW3] = b3

    in_maps = []
    for c in range(NCORE):
        # L1 row shard, pre-transposed + tiled
        W1sh = W1b[c * SH1:(c + 1) * SH1]                    # [1024, 8192]
        w1h = np.ascontiguousarray(
            W1sh.reshape(MB1, 512, KC1, 128).transpose(3, 0, 2, 1)
        ).reshape(128, MB1 * KC1 * 512)
        # L2 column shard
        W2blk = W2b[:, c * SH1:(c + 1) * SH1]                # [8192, 1024]
        w2h = np.ascontiguousarray(
            W2blk.reshape(NB2, 512, KC2, 128).transpose(3, 0, 2, 1)
        ).reshape(128, NB2 * KC2 * 512)
        # L3 row shard of padded W3
        r0 = c * SH3
        pieces = []
        for mb in range(5):
            wmb = MB3W[mb]
            lo = r0 + MB3OFF[mb]
            blk = np.zeros((wmb, SIZE), dtype=bf)
            hi = min(lo + wmb, NROW3)
            if hi > lo:
                blk[:hi - lo] = W3b[lo:hi]
            pieces.append(np.ascontiguousarray(
                blk.reshape(wmb, KC1, 128).transpose(2, 1, 0)
            ).reshape(128, KC1 * wmb))
        w3h = np.concatenate(pieces, axis=1)

        xc = x[c * SAMP_PER_CORE:(c + 1) * SAMP_PER_CORE]
        xgc = np.zeros((128, 8), np.float32)
        xuc = np.zeros((128, NU), np.float32)
        for gidx in range(8):
            seg = xc[gidx * 128:(gidx + 1) * 128]
            xuc[16 * gidx:16 * (gidx + 1), :] = seg[None, :]
            xgc[16 * gidx:16 * gidx + 16, :] = seg.reshape(8, 16).T
        in_maps.append({
            "w1": w1h,
            "w2": w2h,
            "w3": w3h,
            "b1d": np.ascontiguousarray(b1[c * SH1:(c + 1) * SH1]).astype(
                np.float32),
            "b2d": np.ascontiguousarray(
                np.asarray(b2, np.float32).reshape(KC1, 128).T),
            "b3d": np.ascontiguousarray(b3p[c * SH3:(c + 1) * SH3]),
            "xbf": x.astype(bf),
            "xg": xgc,
            "xu": xuc,
            "sjb": np.broadcast_to(s, (128, NJ)).copy(),
            "ab": np.broadcast_to(a.astype(bf), (128, NJ)).copy(),
            "bb": np.broadcast_to(b.astype(bf), (128, NJ)).copy(),
        })
    return in_maps


def kernel(x, Wc, W1, b1, W2, b2, W3, b3, _trace=False, _res_box=None):
    nc = _get_nc()
    in_maps = _host_prep(np.asarray(x), Wc, np.asarray(W1), np.asarray(b1),
                         np.asarray(W2), np.asarray(b2), np.asarray(W3),
                         np.asarray(b3))
    res = run_bass_kernel_spmd(nc, in_maps, core_ids=list(range(NCORE)),
                               trace=_trace)
    if _res_box is not None:
        _res_box.append(res)
    i1 = np.concatenate([r["out"][0:1024] for r in res.results])
    i2 = np.concatenate([r["out"][1024:2048] for r in res.results])
    return np.concatenate([i1, i2]).astype(np.float32)


# revision 3
# speedup vs baseline: 1.2355x; 1.0992x over previous
"""Trainium2 Bass kernel for nn_FCN_81621558493619 (v2: TensorEngine MLP).

Computation: 3-layer MLP (mat-vec, 8192->8192->8192->16394) + box-filter +
linear interpolation + Fermi-window trapezoid integrals.

Strategy (8 NeuronCores, SPMD + collectives):
  - Mat-vecs on the TensorEngine: stationary = x k-chunk [128,1] (LDWEIGHTS
    ~1 cycle), moving = W.T tile [128,512] bf16 streamed from SBUF at
    128 elem/cycle.  PSUM rows [1,512] accumulate over 64 k-chunks.
  - Sharding: L1 row-shard (1024 rows/core), L2 column-shard (local h1
    shard as stationary -> no collective between L1/L2), partial y2
    AllReduce'd in 4 segments (overlapped with L2/L3), L3 row-shard
    (2176 rows/core of zero-padded 17408), AllGather Q.
  - Weights pre-transposed + tiled on host so each DMA chunk is a
    [128, contig] slice (1-2 MB per dma_start, ~line rate).
  - Interpolation stage identical to v1 baseline: bf16 pair table
    T2[m] = (S10[m], S10[m+1]), chunked gpsimd ap_gather (d=2), DVE
    blend + weighted trapz reductions.  Index/frac prep hides under the
    MLP (DVE is otherwise idle now).
"""
import numpy as np
import ml_dtypes

import concourse.bacc as bacc
import concourse.mybir as mybir
from concourse import tile
from concourse.bass_utils import run_bass_kernel_spmd

F32 = mybir.dt.float32
BF16 = mybir.dt.bfloat16
I16 = mybir.dt.int16
I32 = mybir.dt.int32

SIZE = 8192
NCORE = 8
SH1 = SIZE // NCORE            # 1024 rows of W1 (and cols of W2) per core
NROW3 = 2 * SIZE + 10          # 16394
NROW3P = 17408                 # padded to 8*2176
SH3 = NROW3P // NCORE          # 2176
NJ = 101
NJ2 = 51                       # mirror-folded j count (j pairs with 100-j)
NU = 128                       # samples per group
SAMP_PER_CORE = SIZE // NCORE  # 1024
J_CHUNKS = [(0, 13), (13, 26), (26, 39), (39, 51)]
T4N = 8320                     # d4 mirror table entries (i0 <= 8192 for j<=50)

KC1 = 64                       # k-chunks of 128 in L1/L3 contraction
KC2 = 8                        # k-chunks in L2 contraction (local 1024)
MB1 = 2                        # L1 m-blocks of 512
NB2 = 16                       # L2 n-blocks of 512
MB3W = [512, 512, 512, 512, 128]   # L3 m-block widths (2176 total)
MB3OFF = [0, 512, 1024, 1536, 2048]
W3COL = KC1 * SH3              # per-partition w3 row length: 139264
SUBKC = 16                     # k-chunks per DMA chunk (2 MB for 512-wide)


def build_nc():
    nc = bacc.Bacc("TRN2", target_bir_lowering=False, debug=False,
                   num_devices=NCORE)

    # ---- per-core external inputs ----
    # weight layouts (host-pretransposed, see _host_prep):
    #   w1[p, mb*32768 + kc*512 + n] = W1sh[mb*512+n, kc*128+p]
    #   w2[p, nb*4096  + kc*512 + n] = W2blk[nb*512+n, kc*128+p]
    #   w3[p, mboff*64 + kc*wmb + n] = W3sh[mb*512+n, kc*128+p]
    w1 = nc.dram_tensor("w1", [128, MB1 * KC1 * 512], BF16, kind="ExternalInput")
    w2 = nc.dram_tensor("w2", [128, NB2 * KC2 * 512], BF16, kind="ExternalInput")
    w3 = nc.dram_tensor("w3", [128, W3COL], BF16, kind="ExternalInput")
    b1d = nc.dram_tensor("b1d", [SH1], F32, kind="ExternalInput")
    b2d = nc.dram_tensor("b2d", [128, KC1], F32, kind="ExternalInput")
    b3d = nc.dram_tensor("b3d", [SH3], F32, kind="ExternalInput")
    xbf = nc.dram_tensor("xbf", [SIZE], BF16, kind="ExternalInput")
    xg = nc.dram_tensor("xg", [128, 8], F32, kind="ExternalInput")
    xu = nc.dram_tensor("xu", [128, NU], F32, kind="ExternalInput")
    sjb = nc.dram_tensor("sjb", [128, NJ2], F32, kind="ExternalInput")
    ab = nc.dram_tensor("ab", [128, NJ2], BF16, kind="ExternalInput")
    bb = nc.dram_tensor("bb", [128, NJ2], BF16, kind="ExternalInput")
    jex = nc.dram_tensor("jex", [128, 128], BF16, kind="ExternalInput")
    out = nc.dram_tensor("out", [2048], F32, kind="ExternalOutput")

    RG = [list(range(NCORE))]

    with tile.TileContext(nc) as tc:
        with tc.tile_pool(name="dram", bufs=1, space="DRAM") as dpool, \
             tc.tile_pool(name="small", bufs=1) as sp:
            # persistent small tiles
            xgt = sp.tile([128, 8], F32)
            xut = sp.tile([128, NU], F32)
            sjt = sp.tile([128, NJ2], F32)
            abt = sp.tile([128, NJ2], BF16)
            bbt = sp.tile([128, NJ2], BF16)
            i1acc = sp.tile([128, NU], F32)
            i2acc = sp.tile([128, NU], F32)
            idx16 = sp.tile([128, NJ2 * 8], I16)
            frb = sp.tile([128, NJ2 * NU], BF16)  # frac, bf16, (j,u) layout
            jt = sp.tile([128, 128], BF16)        # exchange matrix J
            nc.sync.dma_start(jt[:], jex[:])

            nc.sync.dma_start(xgt[:], xg[:])
            nc.sync.dma_start(xut[:], xu[:])
            nc.sync.dma_start(sjt[:], sjb[:])
            nc.sync.dma_start(abt[:], ab[:])
            nc.sync.dma_start(bbt[:], bb[:])
            nc.vector.memset(i1acc[:], 0.0)
            nc.vector.memset(i2acc[:], 0.0)

            # DRAM bounce buffers
            h1dr = dpool.tile([SH1], BF16, name="h1dr")
            ar_in = [dpool.tile([2048], F32, name=f"ar_in{s}") for s in range(4)]
            ar_out = [dpool.tile([2048], F32, name=f"ar_out{s}") for s in range(4)]
            q_in = dpool.tile([SH3], F32, name="q_in")
            q_full = dpool.tile([NROW3P], F32, name="q_full")
            t4d = dpool.tile([4 * T4N], BF16, name="t4d")

            with tc.tile_pool(name="prep", bufs=1) as pp, \
                 tc.tile_pool(name="mlp_w", bufs=4) as wpool, \
                 tc.tile_pool(name="mlp_m", bufs=1) as mm:
                # MLP-scoped tiles ([1, N] tiles still cost N*dtype bytes of
                # every partition's budget -- keep them out of the
                # persistent pool so the interp pool can fit later)
                xsb1 = mm.tile([128, KC1], BF16)
                xsb2 = mm.tile([128, KC2], BF16)
                xsb3 = mm.tile([128, KC1], BF16)
                b1sb = mm.tile([1, SH1], F32)
                b2sb = mm.tile([128, KC1], F32)
                b3sb = mm.tile([1, SH3], F32)
                h1f = mm.tile([1, SH1], F32)
                h1b = mm.tile([1, SH1], BF16)
                y2sb = mm.tile([1, SIZE], F32)
                qsb = mm.tile([1, SH3], F32)
                nc.sync.dma_start(
                    xsb1[:], xbf.ap().rearrange("(kc p) -> p kc", p=128))
                nc.sync.dma_start(
                    b1sb[:], b1d.ap().rearrange("(o f) -> o f", o=1))
                nc.sync.dma_start(b2sb[:], b2d[:])
                nc.sync.dma_start(
                    b3sb[:], b3d.ap().rearrange("(o f) -> o f", o=1))
                # ------------ index/frac prep (depends only on x) ---------
                for (j0, j1) in J_CHUNKS:
                    jc = j1 - j0
                    wq = jc * NU
                    sx = pp.tile([128, wq], F32, tag="sx", name="sx")
                    pm = pp.tile([128, wq], F32, tag="pm", name="pm")
                    i0i = pp.tile([128, wq], I32, tag="i0i", name="i0i")
                    i0f = pp.tile([128, wq], F32, tag="i0f", name="i0f")
                    # sx[p, (j,u)] = s_j * x_u   (x replicated per group)
                    nc.vector.tensor_tensor(
                        out=sx[:],
                        in0=xut[:].unsqueeze(1).to_broadcast([128, jc, NU]),
                        in1=sjt[:, j0:j1].unsqueeze(2).to_broadcast([128, jc, NU]),
                        op=mybir.AluOpType.mult)
                    nc.vector.tensor_scalar_add(pm[:], sx[:], 8191.5)
                    nc.vector.tensor_copy(i0i[:], pm[:])  # HW cast rounds -> floor
                    # i0f = min(i0, 16383) - 8192  (f32)
                    nc.vector.tensor_scalar(
                        out=i0f[:], in0=i0i[:], scalar1=16383, scalar2=8192,
                        op0=mybir.AluOpType.min, op1=mybir.AluOpType.subtract)
                    # frac = sx - i0f  (bf16)
                    nc.vector.tensor_tensor(
                        out=frb[:, j0 * NU:j1 * NU], in0=sx[:], in1=i0f[:],
                        op=mybir.AluOpType.subtract)

                    # compact index path for the gather
                    wg = jc * 8
                    sxg = pp.tile([128, wg], F32, tag="sxg", name="sxg")
                    pmg = pp.tile([128, wg], F32, tag="pmg", name="pmg")
                    i0g = pp.tile([128, wg], I32, tag="i0g", name="i0g")
                    nc.vector.tensor_tensor(
                        out=sxg[:],
                        in0=xgt[:].unsqueeze(1).to_broadcast([128, jc, 8]),
                        in1=sjt[:, j0:j1].unsqueeze(2).to_broadcast([128, jc, 8]),
                        op=mybir.AluOpType.mult)
                    nc.vector.tensor_scalar_add(pmg[:], sxg[:], 8191.5)
                    nc.vector.tensor_copy(i0g[:], pmg[:])
                    nc.vector.tensor_scalar(
                        out=idx16[:, j0 * 8:j1 * 8], in0=i0g[:], scalar1=8192,
                        scalar2=None, op0=mybir.AluOpType.min)

                # ------------------ L1: h1_shard = relu(W1sh @ x + b1sh) --
                with tc.tile_pool(name="ps1", bufs=1, space="PSUM") as ps1p:
                    ps1 = [ps1p.tile([1, 512], F32, tag=f"ps1_{mb}",
                                     name=f"ps1_{mb}") for mb in range(MB1)]
                    for mb in range(MB1):
                        for sub in range(KC1 // SUBKC):
                            wt = wpool.tile([128, SUBKC * 512], BF16, tag="w",
                                            name=f"w1_{mb}_{sub}")
                            off = mb * KC1 * 512 + sub * SUBKC * 512
                            weng = nc.sync if (mb * 4 + sub) % 2 == 0 \
                                else nc.scalar
                            weng.dma_start(
                                wt[:], w1[:, off:off + SUBKC * 512])
                            for j in range(SUBKC):
                                kc = sub * SUBKC + j
                                nc.tensor.matmul(
                                    ps1[mb][:, :],
                                    xsb1[:, kc:kc + 1],
                                    wt[:, j * 512:(j + 1) * 512],
                                    start=(kc == 0), stop=(kc == KC1 - 1))
                        # per-m-block epilogue: bias + relu -> bf16
                        sl = slice(mb * 512, (mb + 1) * 512)
                        nc.vector.tensor_tensor(
                            out=h1f[:, sl], in0=ps1[mb][:, :], in1=b1sb[:, sl],
                            op=mybir.AluOpType.add)
                        nc.vector.tensor_scalar_max(h1b[:, sl], h1f[:, sl], 0.0)
                        nc.sync.dma_start(
                            h1dr[mb * 512:(mb + 1) * 512].rearrange(
                                "(o f) -> o f", o=1), h1b[:, sl])
                # h1 bf16 [1024] -> stationary layout [128, 8]
                nc.sync.dma_start(
                    xsb2[:], h1dr[:].rearrange("(kc p) -> p kc", p=128))

                # ------------------ L2: y2_partial = W2[:, blk] @ h1_shard --
                with tc.tile_pool(name="ps2", bufs=1, space="PSUM") as ps2p:
                    for nb in range(NB2):
                        wt = wpool.tile([128, KC2 * 512], BF16, tag="w",
                                        name=f"w2_{nb}")
                        off = nb * KC2 * 512
                        weng = nc.sync if nb % 2 == 0 else nc.scalar
                        weng.dma_start(wt[:], w2[:, off:off + KC2 * 512])
                        ps = ps2p.tile([1, 512], F32, tag="ps2", bufs=3,
                                       name=f"ps2_{nb}")
                        for kc in range(KC2):
                            nc.tensor.matmul(
                                ps[:, :], xsb2[:, kc:kc + 1],
                                wt[:, kc * 512:(kc + 1) * 512],
                                start=(kc == 0), stop=(kc == KC2 - 1))
                        nc.scalar.activation(
                            out=y2sb[:, nb * 512:(nb + 1) * 512], in_=ps[:, :],
                            func=mybir.ActivationFunctionType.Copy)
                        if nb % 4 == 3:
                            s = nb // 4
                            nc.sync.dma_start(
                                ar_in[s][:].rearrange("(o f) -> o f", o=1),
                                y2sb[:, s * 2048:(s + 1) * 2048])
                            nc.gpsimd.collective_compute(
                                "AllReduce", mybir.AluOpType.add,
                                replica_groups=RG,
                                ins=[ar_in[s].opt()], outs=[ar_out[s].opt()])
                            # post-AR: + b2, relu -> xsb3 seg (bf16)
                            t32 = pp.tile([128, 16], F32, tag="t32", bufs=2,
                                          name=f"t32_{s}")
                            nc.sync.dma_start(
                                t32[:],
                                ar_out[s][:].rearrange("(kc p) -> p kc",
                                                       p=128))
                            ssl = slice(s * 16, (s + 1) * 16)
                            nc.vector.tensor_tensor(
                                out=t32[:], in0=t32[:], in1=b2sb[:, ssl],
                                op=mybir.AluOpType.add)
                            nc.vector.tensor_scalar_max(
                                xsb3[:, ssl], t32[:], 0.0)

                # ------------------ L3: Q_shard = W3sh @ h2 + b3sh ---------
                with tc.tile_pool(name="ps3", bufs=1, space="PSUM") as ps3p:
                    for mb in range(5):
                        wmb = MB3W[mb]
                        ps = ps3p.tile([1, wmb], F32, tag=f"ps3_{mb}",
                                       name=f"ps3_{mb}")
                        mboff = MB3OFF[mb] * KC1
                        for sub in range(KC1 // SUBKC):
                            wt = wpool.tile([128, SUBKC * wmb], BF16, tag="w",
                                            name=f"w3_{mb}_{sub}")
                            off = mboff + sub * SUBKC * wmb
                            weng = nc.sync if (mb * 4 + sub) % 2 == 0 \
                                else nc.scalar
                            weng.dma_start(
                                wt[:], w3[:, off:off + SUBKC * wmb])
                            for j in range(SUBKC):
                                kc = sub * SUBKC + j
                                nc.tensor.matmul(
                                    ps[:, :], xsb3[:, kc:kc + 1],
                                    wt[:, j * wmb:(j + 1) * wmb],
                                    start=(kc == 0), stop=(kc == KC1 - 1))
                        sl = slice(MB3OFF[mb], MB3OFF[mb] + wmb)
                        nc.vector.tensor_tensor(
                            out=qsb[:, sl], in0=ps[:, :], in1=b3sb[:, sl],
                            op=mybir.AluOpType.add)
                        nc.sync.dma_start(
                            q_in[sl].rearrange("(o f) -> o f", o=1),
                            qsb[:, sl])
                nc.gpsimd.collective_compute(
                    "AllGather", mybir.AluOpType.bypass, replica_groups=RG,
                    ins=[q_in.opt()], outs=[q_full.opt()])

            # ---------------- box sum + pair table ----------------
            with tc.tile_pool(name="sig", bufs=1) as gp:
                qov = gp.tile([128, 144], F32)
                sig = gp.tile([128, 129], F32)
                # partition p holds Q[128p .. 128p+143] (overlapping reads)
                from concourse.ap import AP as _AP
                qf_ap = q_full[:]
                nc.sync.dma_start(
                    qov[:], _AP(qf_ap.tensor, 0, [[128, 128], [1, 144]]))
                nc.vector.tensor_copy(sig[:], qov[:, 0:129])
                for d in range(1, 10):
                    nc.vector.tensor_tensor(out=sig[:], in0=sig[:],
                                            in1=qov[:, d:d + 129],
                                            op=mybir.AluOpType.add)
                # d4 mirror table t4d[m] = (S[m], S[m+1], S[16383-m],
                # S[16384-m]), m in [0, T4N).  The 180-degree rotation
                # R[p,q] = S[16383-128p-q] is built with two PE matmuls
                # against the exchange matrix J (out = lhsT.T @ J reverses
                # the free axis while transposing; applied twice = rot180) --
                # no negative strides, no descriptor shattering.
                sigb = gp.tile([128, 129], BF16)
                nc.vector.tensor_copy(sigb[:], sig[:])
                with tc.tile_pool(name="psig", bufs=1, space="PSUM") as pgp:
                    m1p = pgp.tile([128, 128], F32, tag="m1p")
                    nc.tensor.matmul(m1p[:], sigb[:, 0:128], jt[:],
                                     start=True, stop=True)
                    m1s = gp.tile([128, 128], BF16)
                    nc.vector.tensor_copy(m1s[:], m1p[:])
                    m2p = pgp.tile([128, 128], F32, tag="m2p")
                    nc.tensor.matmul(m2p[:], m1s[:], jt[:],
                                     start=True, stop=True)
                    rr = gp.tile([128, 128], BF16)
                    nc.vector.tensor_copy(rr[:], m2p[:])
                # interleave entries m = 128p + q (p in [0,64]) in SBUF,
                # then one contiguous DMA out + two tiny fix-ups for the
                # partition-shifted c3 column (c3[p,0] = R[p-1,127]).
                t4sb = gp.tile([128, 512], BF16)
                t4v = t4sb[:].rearrange("p (q c) -> p q c", c=4)
                nc.vector.tensor_copy(t4v[0:65, :, 0], sigb[0:65, 0:128])
                nc.vector.tensor_copy(t4v[0:65, :, 1], sigb[0:65, 1:129])
                nc.vector.tensor_copy(t4v[0:65, :, 2], rr[0:65, :])
                nc.vector.tensor_copy(t4v[0:65, 1:128, 3], rr[0:65, 0:127])
                nc.sync.dma_start(
                    t4d[:].rearrange("(p f) -> p f", p=65, f=512),
                    t4sb[0:65, :])
                t4t = t4d[:].tensor
                nc.sync.dma_start(
                    _AP(t4t, 512 + 3, [[512, 64], [1, 1]]),
                    rr[0:64, 127:128])
                nc.sync.dma_start(
                    _AP(t4t, 3, [[1, 1], [1, 1]]), sigb[127:128, 128:129])

            # ---------------- gather + blend + integrate ----------------
            # d4 gather at i0 yields (g0,g1,g2,g3) = (S[i0], S[i0+1],
            # S[16383-i0], S[16384-i0]).  With f = frac:
            #   Sint(j)       = g0(1-f) + g1 f
            #   Sint(100-j)   = g2 f    + g3(1-f)
            #   sum  = (g0+g3) + f*((g1+g2)-(g0+g3))   -> I1 (a_j symmetric)
            #   diff = (g0-g3) + f*((g1-g2)-(g0-g3))   -> I2 (b_j antisym)
            with tc.tile_pool(name="interp", bufs=1) as ip:
                # replicate the d4 table to all partitions: two contiguous
                # broadcasts split across the two HWDGE rings
                tab4 = ip.tile([128, 4 * T4N], BF16)
                HB = 2 * T4N
                nc.sync.dma_start(
                    tab4[:, 0:HB], t4d[0:HB][None, :].to_broadcast([128, HB]))
                nc.scalar.dma_start(
                    tab4[:, HB:], t4d[HB:4 * T4N][None, :]
                    .to_broadcast([128, HB]))
                for ci, (j0, j1) in enumerate(J_CHUNKS):
                    jc = j1 - j0
                    wq = jc * NU
                    gab = ip.tile([128, 4 * wq], BF16, tag="gab", bufs=2,
                                  name=f"gab{ci}")
                    nc.gpsimd.ap_gather(
                        gab[:], tab4[:], idx16[:, j0 * 8:j1 * 8],
                        channels=128, num_elems=T4N, d=4, num_idxs=wq)
                    gv = gab[:].rearrange("p (q c) -> p q c", c=4)
                    g0, g1, g2, g3 = (gv[:, :, k] for k in range(4))
                    d1 = ip.tile([128, wq], BF16, tag="d1", name=f"d1{ci}")
                    d2 = ip.tile([128, wq], BF16, tag="d2", name=f"d2{ci}")
                    ds = ip.tile([128, wq], BF16, tag="ds", name=f"ds{ci}")
                    ps = ip.tile([128, wq], BF16, tag="ps", name=f"ps{ci}")
                    rs = ip.tile([128, wq], BF16, tag="rs", name=f"rs{ci}")
                    i1p = ip.tile([128, NU], F32, tag="i1p", bufs=2,
                                  name=f"i1p{ci}")
                    i2p = ip.tile([128, NU], F32, tag="i2p", bufs=2,
                                  name=f"i2p{ci}")
                    frc = frb[:, j0 * NU:j1 * NU]
                    nc.vector.tensor_tensor(out=d1[:], in0=g1, in1=g0,
                                            op=mybir.AluOpType.subtract)
                    nc.vector.tensor_tensor(out=d2[:], in0=g2, in1=g3,
                                            op=mybir.AluOpType.subtract)
                    nc.vector.tensor_tensor(out=ds[:], in0=d1[:], in1=d2[:],
                                            op=mybir.AluOpType.add)
                    nc.vector.tensor_tensor(out=ds[:], in0=frc, in1=ds[:],
                                            op=mybir.AluOpType.mult)
                    nc.vector.tensor_tensor(out=ps[:], in0=g0, in1=g3,
                                            op=mybir.AluOpType.add)
                    nc.vector.tensor_tensor(out=ps[:], in0=ps[:], in1=ds[:],
                                            op=mybir.AluOpType.add)
                    abv = abt[:, j0:j1].unsqueeze(2).to_broadcast([128, jc, NU])
                    bbv = bbt[:, j0:j1].unsqueeze(2).to_broadcast([128, jc, NU])
                    nc.vector.tensor_tensor(out=ps[:], in0=ps[:], in1=abv,
                                            op=mybir.AluOpType.mult)
                    nc.vector.tensor_reduce(
                        out=i1p[:], in_=ps[:].rearrange("p (j u) -> p u j",
                                                        j=jc, u=NU),
                        axis=mybir.AxisListType.X, op=mybir.AluOpType.add)
                    nc.vector.tensor_tensor(out=i1acc[:], in0=i1acc[:],
                                            in1=i1p[:],
                                            op=mybir.AluOpType.add)
                    nc.vector.tensor_tensor(out=d1[:], in0=d1[:], in1=d2[:],
                                            op=mybir.AluOpType.subtract)
                    nc.vector.tensor_tensor(out=d1[:], in0=frc, in1=d1[:],
                                            op=mybir.AluOpType.mult)
                    nc.vector.tensor_tensor(out=rs[:], in0=g0, in1=g3,
                                            op=mybir.AluOpType.subtract)
                    nc.vector.tensor_tensor(out=rs[:], in0=rs[:], in1=d1[:],
                                            op=mybir.AluOpType.add)
                    nc.vector.tensor_tensor(out=rs[:], in0=rs[:], in1=bbv,
                                            op=mybir.AluOpType.mult)
                    nc.vector.tensor_reduce(
                        out=i2p[:], in_=rs[:].rearrange("p (j u) -> p u j",
                                                        j=jc, u=NU),
                        axis=mybir.AxisListType.X, op=mybir.AluOpType.add)
                    nc.vector.tensor_tensor(out=i2acc[:], in0=i2acc[:],
                                            in1=i2p[:],
                                            op=mybir.AluOpType.add)

                # I2 = x_i * sum_j b_j S_ij  (x does not cancel for I2)
                nc.vector.tensor_tensor(out=i2acc[:], in0=i2acc[:],
                                        in1=xut[:], op=mybir.AluOpType.mult)
                # outputs: row r=0 of each 16-partition group
                nc.sync.dma_start(
                    out[0:1024].rearrange("(g u) -> g u", g=8, u=NU),
                    i1acc[0:128:16, :])
                nc.sync.dma_start(
                    out[1024:2048].rearrange("(g u) -> g u", g=8, u=NU),
                    i2acc[0:128:16, :])

    nc.compile()
    return nc


_NC_CACHE = {}


def _get_nc():
    if "nc" not in _NC_CACHE:
        _NC_CACHE["nc"] = build_nc()
    return _NC_CACHE["nc"]


def _host_prep(x, Wc, W1, b1, W2, b2, W3, b3):
    bf = ml_dtypes.bfloat16
    x = np.asarray(x, np.float32)
    Wcf = np.float64(np.asarray(Wc).item())
    # t grid and Fermi window (match reference's fp32 values closely)
    t = (np.linspace(-1.0, 1.0, NJ, dtype=np.float32)
         * np.float32(Wcf)).astype(np.float32)
    step = np.float32(Wcf) / np.float32(SIZE)
    s = (t / step).astype(np.float32)           # pos = x*s + SIZE
    eu = np.exp(t.astype(np.float64))
    g = eu / (eu + 1.0) ** 2                     # fermi window * x (x cancels)
    d = np.diff(t.astype(np.float64))            # actual fp32 grid deltas
    wtrap = np.zeros(NJ)
    wtrap[:-1] += 0.5 * d
    wtrap[1:] += 0.5 * d
    a = (0.1 * g * wtrap).astype(np.float32)     # 0.1 = box-filter fold
    b = (-0.1 * t.astype(np.float64) * g * wtrap).astype(np.float32)
    # mirror fold: j in [0,50] covers (j, 100-j); j=50 pairs with itself so
    # its a-weight is halved (b[50] is exactly 0 since t[50]=0).
    s = s[:NJ2].copy()
    a = a[:NJ2].copy()
    b = b[:NJ2].copy()
    a[NJ2 - 1] *= 0.5

    W1b = np.asarray(W1, np.float32).astype(bf)
    W2b = np.asarray(W2, np.float32).astype(bf)
    W3b = np.asarray(W3, np.float32).astype(bf)
    b3p = np.zeros(NROW3P, dtype=np.float32)
    b3p[:NROW3] = b3

    in_maps = []
    for c in range(NCORE):
        # L1 row shard, pre-transposed + tiled
        W1sh = W1b[c * SH1:(c + 1) * SH1]                    # [1024, 8192]
        w1h = np.ascontiguousarray(
            W1sh.reshape(MB1, 512, KC1, 128).transpose(3, 0, 2, 1)
        ).reshape(128, MB1 * KC1 * 512)
        # L2 column shard
        W2blk = W2b[:, c * SH1:(c + 1) * SH1]                # [8192, 1024]
        w2h = np.ascontiguousarray(
            W2blk.reshape(NB2, 512, KC2, 128).transpose(3, 0, 2, 1)
        ).reshape(128, NB2 * KC2 * 512)
        # L3 row shard of padded W3
        r0 = c * SH3
        pieces = []
        for mb in range(5):
            wmb = MB3W[mb]
            lo = r0 + MB3OFF[mb]
            blk = np.zeros((wmb, SIZE), dtype=bf)
            hi = min(lo + wmb, NROW3)
            if hi > lo:
                blk[:hi - lo] = W3b[lo:hi]
            pieces.append(np.ascontiguousarray(
                blk.reshape(wmb, KC1, 128).transpose(2, 1, 0)
            ).reshape(128, KC1 * wmb))
        w3h = np.concatenate(pieces, axis=1)

        xc = x[c * SAMP_PER_CORE:(c + 1) * SAMP_PER_CORE]
        xgc = np.zeros((128, 8), np.float32)
        xuc = np.zeros((128, NU), np.float32)
        for gidx in range(8):
            seg = xc[gidx * 128:(gidx + 1) * 128]
            xuc[16 * gidx:16 * (gidx + 1), :] = seg[None, :]
            xgc[16 * gidx:16 * gidx + 16, :] = seg.reshape(8, 16).T
        in_maps.append({
            "w1": w1h,
            "w2": w2h,
            "w3": w3h,
            "b1d": np.ascontiguousarray(b1[c * SH1:(c + 1) * SH1]).astype(
                np.float32),
            "b2d": np.ascontiguousarray(
                np.asarray(b2, np.float32).reshape(KC1, 128).T),
            "b3d": np.ascontiguousarray(b3p[c * SH3:(c + 1) * SH3]),
            "xbf": x.astype(bf),
            "xg": xgc,
            "xu": xuc,
            "sjb": np.broadcast_to(s, (128, NJ2)).copy(),
            "ab": np.broadcast_to(a.astype(bf), (128, NJ2)).copy(),
            "bb": np.broadcast_to(b.astype(bf), (128, NJ2)).copy(),
            "jex": np.eye(128, dtype=np.float32)[::-1].astype(bf).copy(),
        })
    return in_maps


def kernel(x, Wc, W1, b1, W2, b2, W3, b3, _trace=False, _res_box=None):
    nc = _get_nc()
    in_maps = _host_prep(np.asarray(x), Wc, np.asarray(W1), np.asarray(b1),
                         np.asarray(W2), np.asarray(b2), np.asarray(W3),
                         np.asarray(b3))
    res = run_bass_kernel_spmd(nc, in_maps, core_ids=list(range(NCORE)),
                               trace=_trace)
    if _res_box is not None:
        _res_box.append(res)
    i1 = np.concatenate([r["out"][0:1024] for r in res.results])
    i2 = np.concatenate([r["out"][1024:2048] for r in res.results])
    return np.concatenate([i1, i2]).astype(np.float32)
